# revision 24
# baseline (speedup 1.0000x reference)
"""Multi-head attention (B=2, S=2048, D=1024, H=16, hd=64) with RoPE on 8 TRN2
NeuronCores.

Sharding: 2 batches x 4 head-groups. Core c handles batch c//4, heads
[4*(c%4), 4*(c%4)+4). Each core computes Q/K/V projections for its heads from
the full sequence, RoPE, unnormalized attention (exp(q.k/8) streamed through
PSUM with a ones/exp(mask) column appended to V to collect the softmax row
sums), then normalizes. An AllToAll over each batch's 4-core group exchanges
Each core then computes its partial output projection (contraction over its
own 4 heads, all 2048 rows) and a bf16 ReduceScatter over the batch's 4-core
group sums the head-group partials and hands each core a distinct 512-row
slice of the finished projection. The host concatenates the 8 slices and adds
the (wo + wv@wo) bias.

Layout notes:
- x is uploaded pre-transposed (xT [D, S]) so it serves both as matmul rhs for
  Q^T/K^T production and as lhsT for V production.
- Q^T/K^T rows within each head are permuted to (d0,d32,d1,d33,...) so the
  RoPE partner lives in the adjacent partition; a stream_shuffle with the
  pair-swap mask plus two multiplies by host-precomputed cos/sin tables
  implements the rotation with all operands partition-aligned. The score
  matmul contracts over the permuted axis, which is permutation-invariant as
  long as Q and K share the ordering.
- The attention mask enters as exp(mask[k]) multiplied into V's rows (and
  the appended row-sum column), which is exact and free.
"""

import numpy as np
import ml_dtypes

import concourse.bass as bass
import concourse.mybir as mybir
from concourse import bacc, bass_utils
import concourse.tile as tile

B, S, DIM, HEADS, HD = 2, 2048, 1024, 16, 64
HPC = HEADS // 4          # heads per core = 4
P = 128
NKC = DIM // P            # 8 contraction chunks for projections
NSC = S // P              # 16 sequence chunks of 128
NQT = S // 512            # 4 q tiles of 512
SQ = S // 4               # 512-row output slice per core
VW = HPC * (HD + 1)       # 260: V with a row-sum column per head
fp32 = mybir.dt.float32
bf16 = mybir.dt.bfloat16

_CACHE = {}


def _build(dbg=False):
    nc = bacc.Bacc("TRN2", target_bir_lowering=False, debug=False, num_devices=8)

    xT = nc.dram_tensor("xT", [DIM, S], bf16, kind="ExternalInput")
    wq = nc.dram_tensor("wq", [DIM, HPC * HD], bf16, kind="ExternalInput")
    wk = nc.dram_tensor("wk", [DIM, HPC * HD], bf16, kind="ExternalInput")
    wv = nc.dram_tensor("wv", [DIM, HPC * HD], bf16, kind="ExternalInput")
    wo = nc.dram_tensor("wo", [HPC * HD, DIM], bf16, kind="ExternalInput")
    trigA = nc.dram_tensor("trigA", [P, S], bf16, kind="ExternalInput")
    trigB = nc.dram_tensor("trigB", [P, S], bf16, kind="ExternalInput")
    qbias = nc.dram_tensor("qbias", [P, 2], fp32, kind="ExternalInput")
    kbias = nc.dram_tensor("kbias", [P, 2], fp32, kind="ExternalInput")
    em = nc.dram_tensor("em", [P, NSC], fp32, kind="ExternalInput")
    out = nc.dram_tensor("out", [SQ, DIM], bf16, kind="ExternalOutput")
    if dbg:
        dbg_qt = nc.dram_tensor("dbg_qt", [P, 2 * S], bf16, kind="ExternalOutput")
        dbg_kt = nc.dram_tensor("dbg_kt", [P, 2 * S], bf16, kind="ExternalOutput")
        dbg_v = nc.dram_tensor("dbg_v", [P, NSC * VW], bf16, kind="ExternalOutput")
        dbg_ot = nc.dram_tensor("dbg_ot", [P, 2 * S], bf16, kind="ExternalOutput")
        dbg_part = nc.dram_tensor("dbg_part", [S, DIM], bf16, kind="ExternalOutput")
        dbg_rb = nc.dram_tensor("dbg_rb", [HD, 512], fp32, kind="ExternalOutput")
        dbg_rs = nc.dram_tensor("dbg_rs", [1, 512], fp32, kind="ExternalOutput")
        dbg_st = nc.dram_tensor("dbg_st", [HD, 512], bf16, kind="ExternalOutput")

    SWAP_MASK = [i ^ 1 for i in range(32)]

    with tile.TileContext(nc) as tc:
        with (
            tc.tile_pool(name="const", bufs=1) as const,
            tc.tile_pool(name="work", bufs=3) as work,
            tc.tile_pool(name="attp", bufs=3) as attp,
            tc.tile_pool(name="ps_proj", bufs=2, space="PSUM") as ps_proj,
            tc.tile_pool(name="ps_sT", bufs=2, space="PSUM") as ps_sT,
            tc.tile_pool(name="ps_oT", bufs=2, space="PSUM") as ps_oT,
            tc.tile_pool(name="dram", bufs=1, space="DRAM") as dram,
            tc.tile_pool(name="dram_rc", bufs=3, space="DRAM") as dram_rc,
        ):
            # ---- load constants / inputs into SBUF ----
            xT_sb = const.tile([P, NKC, S], bf16)
            nc.sync.dma_start(xT_sb[:], xT.rearrange("(c p) s -> p c s", p=P))
            wq_sb = const.tile([P, NKC, HPC * HD], bf16)
            nc.sync.dma_start(wq_sb[:], wq.rearrange("(c p) m -> p c m", p=P))
            wk_sb = const.tile([P, NKC, HPC * HD], bf16)
            nc.sync.dma_start(wk_sb[:], wk.rearrange("(c p) m -> p c m", p=P))
            wv_sb = const.tile([P, NKC, HPC * HD], bf16)
            nc.sync.dma_start(wv_sb[:], wv.rearrange("(c p) m -> p c m", p=P))
            wo_sb = const.tile([P, 2, DIM], bf16)
            nc.sync.dma_start(wo_sb[:], wo.rearrange("(c p) m -> p c m", p=P))
            trigA_sb = const.tile([P, S], bf16)
            nc.sync.dma_start(trigA_sb[:], trigA[:])
            trigB_sb = const.tile([P, S], bf16)
            nc.sync.dma_start(trigB_sb[:], trigB[:])
            qb_sb = const.tile([P, 2], fp32)
            nc.sync.dma_start(qb_sb[:], qbias[:])
            kb_sb = const.tile([P, 2], fp32)
            nc.sync.dma_start(kb_sb[:], kbias[:])
            em_sb = const.tile([P, NSC], fp32)
            nc.sync.dma_start(em_sb[:], em[:])

            QT_rot = const.tile([P, 2, S], bf16)   # heads 0,1 | 2,3 stacked
            KT_rot = const.tile([P, 2, S], bf16)
            V_aug = const.tile([P, NSC, VW], bf16)  # [s-chunk, 4*(64+1)]

            # ---- Q^T / K^T projections + RoPE ----
            for w_sb, b_sb, dst in ((wq_sb, qb_sb, QT_rot), (wk_sb, kb_sb, KT_rot)):
                for cq in range(2):          # 128-row chunk: heads 2cq, 2cq+1
                    for sc in range(4):      # 512-col s chunk
                        ps = ps_proj.tile([P, 512], fp32, tag="proj")
                        for kc in range(NKC):
                            nc.tensor.matmul(
                                ps[:],
                                w_sb[:, kc, cq * P:(cq + 1) * P],
                                xT_sb[:, kc, sc * 512:(sc + 1) * 512],
                                start=(kc == 0), stop=(kc == NKC - 1),
                            )
                        q_sb = work.tile([P, 512], bf16, tag="q_sb")
                        nc.vector.tensor_scalar_add(q_sb[:], ps[:], b_sb[:, cq:cq + 1])
                        q_sw = work.tile([P, 512], bf16, tag="q_sw")
                        nc.vector.stream_shuffle(q_sw[:], q_sb[:], SWAP_MASK)
                        p1 = work.tile([P, 512], bf16, tag="p1")
                        nc.vector.tensor_mul(
                            p1[:], q_sb[:], trigA_sb[:, sc * 512:(sc + 1) * 512])
                        p2 = work.tile([P, 512], bf16, tag="p2")
                        nc.vector.tensor_mul(
                            p2[:], q_sw[:], trigB_sb[:, sc * 512:(sc + 1) * 512])
                        nc.vector.tensor_add(
                            dst[:, cq, sc * 512:(sc + 1) * 512], p1[:], p2[:])

            # ---- V projection (natural layout, scaled by exp(mask)) ----
            for sc in range(NSC):
                ps = ps_proj.tile([P, HPC * HD], fp32, tag="proj")
                for kc in range(NKC):
                    nc.tensor.matmul(
                        ps[:],
                        xT_sb[:, kc, sc * P:(sc + 1) * P],
                        wv_sb[:, kc, :],
                        start=(kc == 0), stop=(kc == NKC - 1),
                    )
                # per head: columns 0..63 = V * exp(mask), column 64 = exp(mask)
                vdst = V_aug[:, sc, :].rearrange("p (h x) -> p h x", h=HPC)
                nc.vector.tensor_scalar_mul(
                    vdst[:, :, 0:HD],
                    ps[:].rearrange("p (h x) -> p h x", h=HPC),
                    em_sb[:, sc:sc + 1],
                )
                nc.vector.tensor_copy(
                    vdst[:, :, HD:HD + 1],
                    em_sb[:, sc:sc + 1, None].to_broadcast([P, HPC, 1]),
                )

            # ---- attention: per head, per q-tile, stream k in pairs ----
            oT_norm = const.tile([P, 2, S], bf16)   # normalized o^T, heads packed
            for h in range(HPC):
                pbase = 64 * (h % 2)
                hc = h // 2
                for qt in range(NQT):
                    oT = ps_oT.tile([HD + 1, 512], fp32, tag="oT")
                    for kp in range(NSC // 2):
                        sT = ps_sT.tile([P, 2, 512], fp32, tag="sT")
                        for j in range(2):
                            kb = 2 * kp + j
                            nc.tensor.matmul(
                                sT[:, j, :],
                                KT_rot[pbase:pbase + 64, hc, kb * P:(kb + 1) * P],
                                QT_rot[pbase:pbase + 64, hc, qt * 512:(qt + 1) * 512],
                                start=True, stop=True,
                            )
                        at = attp.tile([P, 2, 512], bf16, tag="attnT")
                        nc.scalar.activation(
                            at[:], sT[:], mybir.ActivationFunctionType.Exp,
                            scale=0.125)
                        for j in range(2):
                            kb = 2 * kp + j
                            nc.tensor.matmul(
                                oT[:],
                                V_aug[:, kb, h * (HD + 1):(h + 1) * (HD + 1)],
                                at[:, j, :],
                                start=(kp == 0 and j == 0),
                                stop=(kp == NSC // 2 - 1 and j == 1),
                            )
                    # normalize: rows 0..63 divided by row 64. Engine ops need
                    # matching, 32-aligned partition bases, so the reciprocal
                    # stays at base 64, a 2KB SBUF DMA moves it to partition 0
                    # for the broadcast, and a DMA repacks the result.
                    rs64 = work.tile([HD + 1, 512], fp32, tag="rs64")
                    nc.vector.tensor_copy(rs64[HD:HD + 1, :], oT[HD:HD + 1, :])
                    rc64 = work.tile([HD + 1, 512], fp32, tag="rc64")
                    nc.vector.reciprocal(
                        rc64[HD:HD + 1, :], rs64[HD:HD + 1, :])
                    if dbg and h == 0 and qt == 0:
                        nc.sync.dma_start(dbg_rs[:], rs64[HD:HD + 1, :])
                    rcd = dram_rc.tile([1, 512], fp32, tag="rcd")
                    nc.gpsimd.dma_start(rcd[:], rc64[HD:HD + 1, :])
                    rb = work.tile([HD, 512], fp32, tag="rbcast")
                    rsrc = rcd[0:1, :]
                    nc.gpsimd.dma_start(
                        rb[:],
                        bass.AP(rsrc.tensor, rsrc.offset, [[0, HD], [1, 512]]))
                    stage = work.tile([HD, 512], bf16, tag="stage")
                    nc.vector.tensor_mul(stage[:], oT[0:HD, :], rb[:])
                    if dbg and h == 0 and qt == 0:
                        nc.sync.dma_start(dbg_rb[:], rb[:])
                        nc.sync.dma_start(dbg_st[:], stage[:])
                    nc.sync.dma_start(
                        oT_norm[pbase:pbase + 64, hc, qt * 512:(qt + 1) * 512],
                        stage[:])

            # ---- partial output projection (my 4 heads, all 2048 rows) ----
            cc_in = dram.tile([S, DIM], bf16)
            cc_out = dram.tile([SQ, DIM], bf16)
            for qs in range(NSC):
                o_sb = work.tile([P, DIM], bf16, tag="o_sb")
                for dc in range(2):
                    ps = ps_proj.tile([P, 512], fp32, tag="proj")
                    for c in range(2):
                        nc.tensor.matmul(
                            ps[:],
                            oT_norm[:, c, qs * P:(qs + 1) * P],
                            wo_sb[:, c, dc * 512:(dc + 1) * 512],
                            start=(c == 0), stop=(c == 1),
                        )
                    nc.vector.tensor_copy(o_sb[:, dc * 512:(dc + 1) * 512], ps[:])
                nc.sync.dma_start(cc_in[qs * P:(qs + 1) * P, :], o_sb[:])

            if dbg:
                nc.sync.dma_start(
                    dbg_qt[:], QT_rot[:].rearrange("p c s -> p (c s)"))
                nc.sync.dma_start(
                    dbg_kt[:], KT_rot[:].rearrange("p c s -> p (c s)"))
                nc.sync.dma_start(
                    dbg_v[:], V_aug[:].rearrange("p c s -> p (c s)"))
                nc.sync.dma_start(
                    dbg_ot[:], oT_norm[:].rearrange("p c s -> p (c s)"))
                nc.sync.dma_start(dbg_part[:], cc_in[:])

            # ---- sum head-group partials; each core keeps a 512-row slice ----
            nc.gpsimd.collective_compute(
                "ReduceScatter", mybir.AluOpType.add,
                replica_groups=[[0, 1, 2, 3], [4, 5, 6, 7]],
                ins=[cc_in.opt()], outs=[cc_out.opt()],
            )
            nc.gpsimd.dma_start(out[:], cc_out[:])

    nc.compile()
    return nc


def _host_prep(x, pos, mask, wq_kernel, wq_bias, wk_kernel, wk_bias,
               wv_kernel, wv_bias, wo_kernel, wo_bias):
    """Build per-core in_maps for the 8 cores."""
    perm = np.array([(j // 2) if j % 2 == 0 else (j // 2 + 32)
                     for j in range(HD)])
    half = HD // 2
    freqs = (10000.0 ** (-np.linspace(0.0, 1.0, half, endpoint=False))).astype(np.float64)

    bf = ml_dtypes.bfloat16
    in_maps = []
    for c in range(8):
        b, g = c // 4, c % 4
        H = list(range(HPC * g, HPC * g + HPC))

        theta = pos[b].astype(np.float64)[:, None] * freqs[None, :]  # [S, 32]
        cos = np.cos(theta).astype(np.float32)
        sin = np.sin(theta).astype(np.float32)
        trigA = np.empty((P, S), np.float32)
        trigB = np.empty((P, S), np.float32)
        for r in range(P):
            j = r % HD
            i = j // 2
            trigA[r] = cos[:, i]
            trigB[r] = (-sin[:, i]) if j % 2 == 0 else sin[:, i]

        def permute_w(wk_):  # [D, H, hd] -> [D, 4*64] with rope-pair row order
            wsel = wk_[:, H, :][:, :, perm]          # [D, 4, 64]
            return np.ascontiguousarray(wsel.reshape(DIM, HPC * HD))

        def permute_b(bias):  # [H, hd] -> [128, 2]
            bsel = bias[H][:, perm]                  # [4, 64]
            return np.ascontiguousarray(bsel.reshape(2, P).T)

        emv = np.exp(mask[b, 0, 0].astype(np.float32))  # [S]

        in_maps.append({
            "xT": np.ascontiguousarray(x[b].T).astype(bf),
            "wq": permute_w(wq_kernel).astype(bf),
            "wk": permute_w(wk_kernel).astype(bf),
            "wv": np.ascontiguousarray(
                wv_kernel[:, H, :].reshape(DIM, HPC * HD)).astype(bf),
            "wo": np.ascontiguousarray(
                wo_kernel[H].reshape(HPC * HD, DIM)).astype(bf),
            "trigA": trigA.astype(bf),
            "trigB": trigB.astype(bf),
            "qbias": permute_b(wq_bias),
            "kbias": permute_b(wk_bias),
            "em": np.ascontiguousarray(emv.reshape(NSC, P).T),
        })
    return in_maps


def kernel(x, pos, mask, wq_kernel, wq_bias, wk_kernel, wk_bias,
           wv_kernel, wv_bias, wo_kernel, wo_bias):
    if "nc" not in _CACHE:
        _CACHE["nc"] = _build()
    nc = _CACHE["nc"]

    in_maps = _host_prep(x, pos, mask, wq_kernel, wq_bias, wk_kernel, wk_bias,
                         wv_kernel, wv_bias, wo_kernel, wo_bias)
    res = bass_utils.run_bass_kernel_spmd(
        nc, in_maps, core_ids=list(range(8)))

    final_bias = (wo_bias.astype(np.float64)
                  + np.einsum("hd,hdo->o", wv_bias.astype(np.float64),
                              wo_kernel.astype(np.float64))).astype(np.float32)

    outs = []
    for b in range(B):
        rows = np.concatenate(
            [np.asarray(res.results[4 * b + j]["out"]).astype(np.float32)
             for j in range(4)], axis=0)
        outs.append(rows + final_bias[None, :])
    return np.stack(outs, axis=0)
